# revision 30
# baseline (speedup 1.0000x reference)
# BEiT-style windowed attention (B=64, N=197, C=768, H=12) on 8 Trainium2
# NeuronCores, data-parallel over batch (8 batches per core).
#
# Single interleaved pipeline per core (no phase barrier): qkv projection
# matmul groups, attention subgroups, and output-projection chunks are
# emitted round-robin so the ACT/DVE softmax work overlaps the dense
# matmuls and the PE is the only near-saturated engine.
#
#   round r (part A): v batches 2r, 2r+1 (r<4); q_r/k_r chunk-half A
#     (token cols 0:788 = batches 0-3); attention subgroups (b<4, j<=r)
#     as they become ready.
#   part B rounds: q_r/k_r chunk-half B (batches 4-7); remaining
#     subgroups; proj chunks per batch-pair as pairs complete.
#
# Attention subgroup (b, head-pair j): S.T = k.T q (2 heads row-tiled
# concurrently in the PE array), exp on ACT, u2 = exp(S)*exp(bias) on
# DVE (bias table sent as exp(bias) from host), then P@V col-tiled
# (head i -> psum partitions 64i:64i+64, free 0:197) plus an
# all-ones-lhsT matmul that writes the softmax denominator broadcast
# across partitions (free 256:453) in the same bank; DVE
# reciprocal_approx_fast [128,197] + one tensor_mul -> OT.
#
# PSUM: qkv/proj groups share one 2-buf single-bank pool (2), psS 2x2
# banks (4), psOD 2x1 (2) = 8 banks.
#
# Host side shards/transposes inputs, gathers exp(rel_table[rel_index]),
# scales q by SCALE (folded into W1/q_bias), and unshards the output.
# v_bias and proj_b are exact host-side constant adds (softmax rows sum
# to 1).

import numpy as np
import ml_dtypes

BF16 = ml_dtypes.bfloat16

DIM = 768
H = 12
HD = 64
NTOK = 197
B = 64
NCORES = 8
BL = B // NCORES          # batches per core = 8
T = BL * NTOK             # 1576 tokens per core
SCALE = HD ** -0.5
CH = 394                  # free-dim chunk for the dense matmuls (4*394 = 1576)
KT0, KT1 = 128, NTOK - 128   # key-token tile sizes (128, 69)
VCH = 384                 # v output-channel chunk (2*384 = 768)

_cache = {}


def _emit(nc):
    import concourse.mybir as mybir
    import concourse.tile as tile

    f32 = mybir.dt.float32
    bf16 = mybir.dt.bfloat16
    AF = mybir.ActivationFunctionType

    xT_d = nc.declare_dram_parameter("xT", [DIM, T], bf16, isOutput=False)
    w1_d = nc.declare_dram_parameter("w1", [DIM, 3 * DIM], bf16, isOutput=False)
    qkvb_d = nc.declare_dram_parameter("qkvb", [128, 12], f32, isOutput=False)
    bT_d = nc.declare_dram_parameter("bT", [128, H, 2 * NTOK], bf16, isOutput=False)
    w2_d = nc.declare_dram_parameter("w2", [DIM, DIM], bf16, isOutput=False)
    yT_d = nc.declare_dram_parameter("yT", [DIM, T], f32, isOutput=True)

    with tile.TileContext(nc) as tc:
        with (
            tc.tile_pool(name="const", bufs=1) as cpool,
            tc.tile_pool(name="qk", bufs=1) as qkpool,
            tc.tile_pool(name="vn", bufs=1) as vpool,
            tc.tile_pool(name="ot", bufs=1) as otpool,
            tc.tile_pool(name="xw", bufs=1) as xw,
            tc.tile_pool(name="p1", bufs=2, space="PSUM") as pp1,
            tc.tile_pool(name="pS", bufs=2, space="PSUM") as pS,
            tc.tile_pool(name="pOD", bufs=2, space="PSUM") as pOD,
            tc.tile_pool(name="u2", bufs=4) as upool,
            tc.tile_pool(name="rn", bufs=4) as rnpool,
            tc.tile_pool(name="yst", bufs=4) as ypool,
        ):
            ones = cpool.tile([128, 128], bf16, tag="ones")
            nc.gpsimd.memset(ones[:], 1.0)
            qkvb = cpool.tile([128, 12], f32, tag="qkvb")
            bT = cpool.tile([128, H, 2 * NTOK], bf16, tag="bT")
            w2sb = [cpool.tile([128, DIM], bf16, name=f"w2_{i}",
                               tag=f"w2_{i}") for i in range(6)]

            # q,k channels-on-partition, split by token half so early
            # batches' attention doesn't wait on late chunks
            qkA = [qkpool.tile([128, 2 * CH], bf16, name=f"qka{i}",
                               tag=f"qka{i}") for i in range(12)]
            qkB = [qkpool.tile([128, 2 * CH], bf16, name=f"qkb{i}",
                               tag=f"qkb{i}") for i in range(12)]
            # v in natural layout per (batch, key-tile): [kt, head, 64]
            vn = [[vpool.tile([128, H, HD], bf16, name=f"vn{b}_{k}",
                             tag=f"vn{b}_{k}")
                   for k in range(2)] for b in range(BL)]
            OT = [otpool.tile([128, T], bf16, name=f"ot{i}", tag=f"ot{i}")
                  for i in range(6)]

            xA = [xw.tile([128, 2 * CH], bf16, name=f"xa{i}",
                          tag=f"xa{i}") for i in range(6)]
            xB = [xw.tile([128, 2 * CH], bf16, name=f"xb{i}",
                          tag=f"xb{i}") for i in range(6)]
            w1q = [xw.tile([128, DIM], bf16, name=f"w1q{i}",
                           tag=f"w1q{i}") for i in range(6)]
            w1k = [xw.tile([128, DIM], bf16, name=f"w1k{i}",
                           tag=f"w1k{i}") for i in range(6)]
            w1v = [xw.tile([128, DIM], bf16, name=f"w1v{i}",
                           tag=f"w1v{i}") for i in range(6)]

            def xtok(kt, lo, hi):
                # x slice for token range [lo, hi) (never straddles 788)
                if hi <= 2 * CH:
                    return xA[kt][:, lo:hi]
                return xB[kt][:, lo - 2 * CH:hi - 2 * CH]

            # DMAs in first-use order: v batches run first (x half A +
            # w1 v-cols), then q0/k0 (w1 q/k cols), exp-bias table, x
            # half B, proj weights.
            qs = [nc.sync, nc.scalar, nc.gpsimd]
            nc.sync.dma_start(out=qkvb[:], in_=qkvb_d[:])
            for i in range(6):
                qs[i % 3].dma_start(out=xA[i][:],
                                    in_=xT_d[128 * i:128 * (i + 1), 0:2 * CH])
            for i in range(6):
                qs[i % 3].dma_start(out=w1v[i][:],
                                    in_=w1_d[128 * i:128 * (i + 1),
                                             2 * DIM:3 * DIM])
            for i in range(6):
                qs[i % 3].dma_start(out=w1q[i][:],
                                    in_=w1_d[128 * i:128 * (i + 1), 0:DIM])
            for i in range(6):
                qs[i % 3].dma_start(out=w1k[i][:],
                                    in_=w1_d[128 * i:128 * (i + 1),
                                             DIM:2 * DIM])
            nc.scalar.dma_start(out=bT[:], in_=bT_d[:])
            for i in range(6):
                qs[i % 3].dma_start(out=xB[i][:],
                                    in_=xT_d[128 * i:128 * (i + 1), 2 * CH:T])
            for i in range(6):
                qs[i % 3].dma_start(
                    out=w2sb[i][:], in_=w2_d[128 * i:128 * (i + 1), :])

            # PE warm-up: matmuls on a zeroed tile keep the HAM activity
            # window busy until real data lands (first v matmul ~8us), so
            # the pipeline starts at 2.4GHz instead of 1.2GHz.
            wt = xw.tile([128, 512], bf16, tag="warm")
            nc.vector.memset(wt[:], 0.0)
            wps = pp1.tile([128, 512], f32, tag="p1", name="wps")
            for _ in range(28):
                nc.tensor.matmul(wps[:, 0:512], ones[:], wt[:],
                                 start=True, stop=True,
                                 skip_group_check=True)
            # dummy exp: pull the exp_and_others ACT table load (~2.7us)
            # forward, out of the first attention subgroup
            wx = xw.tile([1, 8], f32, tag="warmx")
            nc.vector.memset(wx[:], 0.0)
            wy = xw.tile([1, 8], f32, tag="warmy")
            nc.scalar.activation(wy[:], wx[:], AF.Exp)

            nev = [0]
            nsg = [0]

            def qk_group(ot_, half):
                # one o-tile (q_j or k_j), one token half (2 chunks)
                w1o = w1q if ot_ < 6 else w1k
                c0 = 128 * (ot_ % 6)
                dstt = (qkA if half == 0 else qkB)[ot_]
                for cc in range(2):
                    ch = 2 * half + cc
                    ps = pp1.tile([128, 512], f32, tag="p1", name="p1")
                    for kt in range(6):
                        nc.tensor.matmul(
                            ps[:, 0:CH],
                            w1o[kt][:, c0:c0 + 128],
                            xtok(kt, CH * ch, CH * (ch + 1)),
                            start=(kt == 0), stop=(kt == 5),
                        )
                    dst = dstt[:, CH * cc:CH * (cc + 1)]
                    if nev[0] % 2 == 0:
                        nc.scalar.activation(dst, ps[:, 0:CH], AF.Identity,
                                             bias=qkvb[:, ot_:ot_ + 1])
                    else:
                        nc.vector.tensor_scalar_add(dst, ps[:, 0:CH],
                                                    qkvb[:, ot_:ot_ + 1])
                    nev[0] += 1

            def emit_v(b):
                t0 = NTOK * b
                for k in range(2):
                    m = KT0 if k == 0 else KT1
                    ts_ = t0 + 128 * k
                    for c2 in range(2):
                        ps = pp1.tile([128, 512], f32, tag="p1", name="p1")
                        for kt in range(6):
                            nc.tensor.matmul(
                                ps[0:m, 0:VCH],
                                xtok(kt, ts_, ts_ + m),
                                w1v[kt][:, VCH * c2:VCH * (c2 + 1)],
                                start=(kt == 0), stop=(kt == 5),
                            )
                        src_ = ps[0:m, 0:VCH].rearrange("p (a b) -> p a b",
                                                        a=6)
                        dst = vn[b][k][0:m, 6 * c2:6 * (c2 + 1), 0:64]
                        if nev[0] % 2 == 0:
                            nc.scalar.activation(dst, src_, AF.Copy)
                        else:
                            nc.vector.tensor_copy(dst, src_)
                        nev[0] += 1

            def attn(b, j):
                # one attention subgroup: head pair (2j, 2j+1) of batch b
                qk = qkA if b < 4 else qkB
                t0_ = NTOK * b - (0 if b < 4 else 2 * CH)
                tg_ = NTOK * b
                pair = (2 * j, 2 * j + 1)
                psS = pS.tile([128, 2, 512], f32, tag="psS", name="psS")
                for i, h in enumerate(pair):
                    r0 = 64 * i
                    q_ap = qk[j][r0:r0 + 64, t0_:t0_ + NTOK]
                    nc.tensor.matmul(
                        psS[:, i, 0:NTOK],
                        qk[6 + j][r0:r0 + 64, t0_:t0_ + KT0],
                        q_ap,
                        start=True, stop=False, skip_group_check=True,
                    )
                    nc.tensor.matmul(
                        psS[0:KT1, i, NTOK:2 * NTOK],
                        qk[6 + j][r0:r0 + 64, t0_ + KT0:t0_ + NTOK],
                        q_ap,
                        start=False, stop=True, skip_group_check=True,
                    )
                u2e = upool.tile([128, 2, 2 * NTOK], bf16, tag="u2e",
                                 name="u2e")
                nc.scalar.activation(u2e[:], psS[:, :, 0:2 * NTOK], AF.Exp)
                u2 = upool.tile([128, 2, 2 * NTOK], bf16, tag="u2", name="u2")
                nc.vector.tensor_mul(u2[:], u2e[:], bT[:, 2 * j:2 * j + 2, :])
                psOD = pOD.tile([128, 512], f32, tag="psOD", name="psOD")
                # one bank: P@V head i -> partitions 64i:64i+64 free 0:197,
                # denominator (ones-lhsT, broadcast over partitions) at free
                # 256:453.  start=True marks the full bank row pending-zero
                # for the matmul's OWN partition range only, so the first
                # matmul of each 64-partition range carries start=True.
                for i, h in enumerate(pair):
                    nc.tensor.matmul(
                        psOD[64 * i:64 * i + 64, 0:NTOK],
                        vn[b][0][:, h, :],
                        u2[:, i, 0:NTOK],
                        start=True, stop=False, skip_group_check=True,
                    )
                for i in range(2):
                    nc.tensor.matmul(
                        psOD[64 * i:64 * i + 64, 256:256 + NTOK],
                        ones[:, 64 * i:64 * i + 64],
                        u2[:, i, 0:NTOK],
                        start=False, stop=False, skip_group_check=True,
                    )
                for i, h in enumerate(pair):
                    nc.tensor.matmul(
                        psOD[64 * i:64 * i + 64, 0:NTOK],
                        vn[b][1][0:KT1, h, :],
                        u2[0:KT1, i, NTOK:2 * NTOK],
                        start=False, stop=False, skip_group_check=True,
                    )
                for i in range(2):
                    nc.tensor.matmul(
                        psOD[64 * i:64 * i + 64, 256:256 + NTOK],
                        ones[0:KT1, 64 * i:64 * i + 64],
                        u2[0:KT1, i, NTOK:2 * NTOK],
                        start=False, stop=(i == 1), skip_group_check=True,
                    )
                rn = rnpool.tile([128, NTOK], f32, tag="rn", name="rn")
                nc.vector.reciprocal_approx_fast(
                    out=rn[:], in_=psOD[:, 256:256 + NTOK])
                nc.vector.tensor_mul(
                    OT[j][:, tg_:tg_ + NTOK], psOD[:, 0:NTOK], rn[:])
                nsg[0] += 1

            def proj_pair(pb):
                # one 394-wide column chunk (= batch pair (2pb, 2pb+1)) of
                # the projection
                c0 = 2 * NTOK * pb
                for co in range(6):
                    ps = pp1.tile([128, 512], f32, tag="p1", name="p1")
                    for ci in range(6):
                        nc.tensor.matmul(
                            ps[:, 0:2 * NTOK],
                            w2sb[ci][:, 128 * co:128 * co + 128],
                            OT[ci][:, c0:c0 + 2 * NTOK],
                            start=(ci == 0), stop=(ci == 5),
                        )
                    yst = ypool.tile([128, 2 * NTOK], f32, tag="yst",
                                     name="yst")
                    if (co + pb) % 2 == 0:
                        nc.scalar.activation(yst[:], ps[:, 0:2 * NTOK],
                                             AF.Copy)
                    else:
                        nc.vector.tensor_copy(yst[:], ps[:, 0:2 * NTOK])
                    nc.sync.dma_start(
                        out=yT_d[128 * co:128 * (co + 1), c0:c0 + 2 * NTOK],
                        in_=yst[:],
                    )

            # ---------------- interleaved schedule ----------------
            partA = [
                [(0, 0), (1, 0)],
                [(2, 0), (3, 0), (0, 1), (1, 1)],
                [(2, 1), (3, 1), (0, 2), (1, 2)],
                [(2, 2), (3, 2), (0, 3), (1, 3)],
                [(2, 3), (3, 3), (0, 4), (1, 4)],
                [(2, 4), (3, 4), (0, 5), (1, 5)],
            ]
            for r in range(6):
                # vn[4..7] is first read in part B (~halfway through), so
                # those v batches move past round 2 -- round 2 then has
                # weight-only work while the x half-B DMA is still landing.
                # In round 1 the qk groups sit between the two v batches so
                # v3's psum-buffer wait (copy stuck behind exps in the ACT
                # FIFO) is covered by weight-only matmuls.
                if r in (0, 1, 3, 4):
                    vb0 = 2 * r if r < 2 else 2 * (r - 1)
                    emit_v(vb0)
                    if r != 1:
                        emit_v(vb0 + 1)
                    qk_group(r, 0)
                    qk_group(6 + r, 0)
                    if r == 1:
                        emit_v(vb0 + 1)
                else:
                    qk_group(r, 0)
                    qk_group(6 + r, 0)
                for b_, j_ in partA[r]:
                    attn(b_, j_)
            partB = [
                [(2, 5), (3, 5), (4, 0), (5, 0)],
                [(6, 0), (7, 0), (4, 1), (5, 1)],
                [(6, 1), (7, 1), (4, 2), (5, 2)],
                [(6, 2), (7, 2), (4, 3), (5, 3)],
                [(6, 3), (7, 3), (4, 4), (5, 4)],
                [(6, 4), (7, 4), (4, 5), (5, 5)],
            ]
            for r in range(6):
                qk_group(r, 1)
                qk_group(6 + r, 1)
                for b_, j_ in partB[r]:
                    attn(b_, j_)
                if r == 0:
                    proj_pair(0)
                elif r == 1:
                    proj_pair(1)
            proj_pair(2)
            attn(6, 5)
            attn(7, 5)
            proj_pair(3)
    return nc


def build_nc():
    if "nc" not in _cache:
        from concourse import bacc
        nc = bacc.Bacc(None, target_bir_lowering=False, debug=False)
        _emit(nc)
        nc.compile()
        _cache["nc"] = nc
    return _cache["nc"]


def host_prep(x, qkv_w, q_bias, v_bias, rel_table, proj_w, proj_b, rel_index):
    """Shard + lay out inputs for the 8 cores. Returns list of in_maps."""
    x = np.asarray(x, np.float32)
    qkv_w = np.asarray(qkv_w, np.float32)
    q_bias = np.asarray(q_bias, np.float32)
    rel_table = np.asarray(rel_table, np.float32)
    rel_index = np.asarray(rel_index)

    sv = np.ones((3 * DIM, 1), np.float32)
    sv[:DIM] = SCALE
    w1 = np.ascontiguousarray((qkv_w * sv).T).astype(BF16)        # (768, 2304)
    # per-partition bias for the q,k o-tiles (k bias is zero by construction;
    # v_bias is added host-side: softmax rows sum to 1)
    qb = np.concatenate([q_bias * SCALE, np.zeros(DIM, np.float32)])
    qkvb = np.ascontiguousarray(qb.reshape(12, 128).T).astype(np.float32)

    bias = rel_table[rel_index]                # (197, 197, H), [q, k, h]
    BT = np.exp(bias.transpose(2, 1, 0))       # exp(bias): (H, k, q)
    bTdev = np.ones((128, H, 2 * NTOK), np.float32)
    bTdev[:, :, 0:NTOK] = BT.transpose(1, 0, 2)[0:128]
    bTdev[0:KT1, :, NTOK:2 * NTOK] = BT.transpose(1, 0, 2)[128:NTOK]
    bTdev = bTdev.astype(BF16)

    w2 = np.ascontiguousarray(proj_w.T).astype(BF16)              # (768, 768)

    in_maps = []
    for c in range(NCORES):
        xl = x[BL * c:BL * (c + 1)].reshape(T, DIM)
        xTc = np.ascontiguousarray(xl.T).astype(BF16)
        in_maps.append({
            "xT": xTc, "w1": w1, "qkvb": qkvb, "bT": bTdev, "w2": w2,
        })
    return in_maps


def run_device(in_maps, trace=False, tmpdir=None):
    from concourse.bass_utils import run_bass_kernel_spmd
    nc = build_nc()
    res = run_bass_kernel_spmd(
        nc, in_maps, core_ids=list(range(NCORES)), trace=trace, tmpdir=tmpdir
    )
    return res


def kernel(x, qkv_w, q_bias, v_bias, rel_table, proj_w, proj_b, rel_index):
    in_maps = host_prep(x, qkv_w, q_bias, v_bias, rel_table, proj_w, proj_b,
                        rel_index)
    res = run_device(in_maps)
    y = np.empty((B, NTOK, DIM), np.float32)
    for c in range(NCORES):
        yTc = res.results[c]["yT"]
        y[BL * c:BL * (c + 1)] = yTc.T.reshape(BL, NTOK, DIM)
    # exact host-side constant terms: attn rows sum to 1, so v_bias maps to
    # a constant (v_bias @ proj_w.T); proj_b is a plain add.
    v_bias = np.asarray(v_bias, np.float32)
    proj_b = np.asarray(proj_b, np.float32)
    const = proj_b.copy()
    if np.any(v_bias):
        const = const + v_bias @ np.asarray(proj_w, np.float32).T
    if np.any(const):
        y += const
    return y


# revision 31
# speedup vs baseline: 1.1921x; 1.1921x over previous
# BEiT-style windowed attention (B=64, N=197, C=768, H=12) on 8 Trainium2
# NeuronCores, data-parallel over batch (8 batches per core).
#
# Single interleaved pipeline per core (no phase barrier): qkv projection
# matmul groups, attention subgroups, and output-projection chunks are
# emitted round-robin so the ACT/DVE softmax work overlaps the dense
# matmuls and the PE is the only near-saturated engine.
#
#   round r (part A): v batches 2r, 2r+1 (r<4); q_r/k_r chunk-half A
#     (token cols 0:788 = batches 0-3); attention subgroups (b<4, j<=r)
#     as they become ready.
#   part B rounds: q_r/k_r chunk-half B (batches 4-7); remaining
#     subgroups; proj chunks per batch-pair as pairs complete.
#
# Attention subgroup (b, head-pair j): S.T = k.T q (2 heads row-tiled
# concurrently in the PE array), exp on ACT, u2 = exp(S)*exp(bias) on
# DVE (bias table sent as exp(bias) from host), then P@V col-tiled
# (head i -> psum partitions 64i:64i+64, free 0:197) plus an
# all-ones-lhsT matmul that writes the softmax denominator broadcast
# across partitions (free 256:453) in the same bank; DVE
# reciprocal_approx_fast [128,197] + one tensor_mul -> OT.
#
# PSUM: qkv/proj groups share one 2-buf single-bank pool (2), psS 2x2
# banks (4), psOD 2x1 (2) = 8 banks.
#
# Host side shards/transposes inputs, gathers exp(rel_table[rel_index]),
# scales q by SCALE (folded into W1/q_bias), and unshards the output.
# v_bias and proj_b are exact host-side constant adds (softmax rows sum
# to 1).

import numpy as np
import ml_dtypes

BF16 = ml_dtypes.bfloat16

DIM = 768
H = 12
HD = 64
NTOK = 197
B = 64
NCORES = 8
BL = B // NCORES          # batches per core = 8
T = BL * NTOK             # 1576 tokens per core
SCALE = HD ** -0.5
CH = 394                  # free-dim chunk for the dense matmuls (4*394 = 1576)
KT0, KT1 = 128, NTOK - 128   # key-token tile sizes (128, 69)
VCH = 384                 # v output-channel chunk (2*384 = 768)

_cache = {}


def _emit(nc):
    import concourse.mybir as mybir
    import concourse.tile as tile

    f32 = mybir.dt.float32
    bf16 = mybir.dt.bfloat16
    AF = mybir.ActivationFunctionType

    xT_d = nc.declare_dram_parameter("xT", [DIM, T], bf16, isOutput=False)
    w1_d = nc.declare_dram_parameter("w1", [DIM, 3 * DIM], bf16, isOutput=False)
    qkvb_d = nc.declare_dram_parameter("qkvb", [128, 12], f32, isOutput=False)
    bT_d = nc.declare_dram_parameter("bT", [128, H, 2 * NTOK], bf16, isOutput=False)
    w2_d = nc.declare_dram_parameter("w2", [DIM, DIM], bf16, isOutput=False)
    yT_d = nc.declare_dram_parameter("yT", [DIM, T], f32, isOutput=True)

    with tile.TileContext(nc) as tc:
        with (
            tc.tile_pool(name="const", bufs=1) as cpool,
            tc.tile_pool(name="qk", bufs=1) as qkpool,
            tc.tile_pool(name="vn", bufs=1) as vpool,
            tc.tile_pool(name="ot", bufs=1) as otpool,
            tc.tile_pool(name="xw", bufs=1) as xw,
            tc.tile_pool(name="p1", bufs=2, space="PSUM") as pp1,
            tc.tile_pool(name="pS", bufs=2, space="PSUM") as pS,
            tc.tile_pool(name="pOD", bufs=2, space="PSUM") as pOD,
            tc.tile_pool(name="u2", bufs=4) as upool,
            tc.tile_pool(name="rn", bufs=4) as rnpool,
            tc.tile_pool(name="yst", bufs=4) as ypool,
        ):
            ones = cpool.tile([128, 128], bf16, tag="ones")
            nc.gpsimd.memset(ones[:], 1.0)
            qkvb = cpool.tile([128, 12], f32, tag="qkvb")
            bT = cpool.tile([128, H, 2 * NTOK], bf16, tag="bT")
            w2sb = [cpool.tile([128, DIM], bf16, name=f"w2_{i}",
                               tag=f"w2_{i}") for i in range(6)]

            # q,k channels-on-partition, split by token half so early
            # batches' attention doesn't wait on late chunks
            qkA = [qkpool.tile([128, 2 * CH], bf16, name=f"qka{i}",
                               tag=f"qka{i}") for i in range(12)]
            qkB = [qkpool.tile([128, 2 * CH], bf16, name=f"qkb{i}",
                               tag=f"qkb{i}") for i in range(12)]
            # v in natural layout per (batch, key-tile): [kt, head, 64]
            vn = [[vpool.tile([128, H, HD], bf16, name=f"vn{b}_{k}",
                             tag=f"vn{b}_{k}")
                   for k in range(2)] for b in range(BL)]
            OT = [otpool.tile([128, T], bf16, name=f"ot{i}", tag=f"ot{i}")
                  for i in range(6)]

            xA = [xw.tile([128, 2 * CH], bf16, name=f"xa{i}",
                          tag=f"xa{i}") for i in range(6)]
            xB = [xw.tile([128, 2 * CH], bf16, name=f"xb{i}",
                          tag=f"xb{i}") for i in range(6)]
            w1q = [xw.tile([128, DIM], bf16, name=f"w1q{i}",
                           tag=f"w1q{i}") for i in range(6)]
            w1k = [xw.tile([128, DIM], bf16, name=f"w1k{i}",
                           tag=f"w1k{i}") for i in range(6)]
            w1v = [xw.tile([128, DIM], bf16, name=f"w1v{i}",
                           tag=f"w1v{i}") for i in range(6)]

            def xtok(kt, lo, hi):
                # x slice for token range [lo, hi) (never straddles 788)
                if hi <= 2 * CH:
                    return xA[kt][:, lo:hi]
                return xB[kt][:, lo - 2 * CH:hi - 2 * CH]

            # DMAs in first-use order: v batches run first (x half A +
            # w1 v-cols), then q0/k0 (w1 q/k cols), exp-bias table, x
            # half B, proj weights.
            qs = [nc.sync, nc.scalar, nc.gpsimd]
            nc.sync.dma_start(out=qkvb[:], in_=qkvb_d[:])
            for i in range(6):
                qs[i % 3].dma_start(out=xA[i][:],
                                    in_=xT_d[128 * i:128 * (i + 1), 0:2 * CH])
            for i in range(6):
                qs[i % 3].dma_start(out=w1v[i][:],
                                    in_=w1_d[128 * i:128 * (i + 1),
                                             2 * DIM:3 * DIM])
            for i in range(6):
                qs[i % 3].dma_start(out=w1q[i][:],
                                    in_=w1_d[128 * i:128 * (i + 1), 0:DIM])
            for i in range(6):
                qs[i % 3].dma_start(out=w1k[i][:],
                                    in_=w1_d[128 * i:128 * (i + 1),
                                             DIM:2 * DIM])
            nc.scalar.dma_start(out=bT[:], in_=bT_d[:])
            for i in range(6):
                qs[i % 3].dma_start(out=xB[i][:],
                                    in_=xT_d[128 * i:128 * (i + 1), 2 * CH:T])
            for i in range(6):
                qs[i % 3].dma_start(
                    out=w2sb[i][:], in_=w2_d[128 * i:128 * (i + 1), :])

            # PE warm-up: matmuls on a zeroed tile keep the HAM activity
            # window busy until real data lands (first v matmul ~8us), so
            # the pipeline starts at 2.4GHz instead of 1.2GHz.
            wt = xw.tile([128, 512], bf16, tag="warm")
            nc.vector.memset(wt[:], 0.0)
            wps = pp1.tile([128, 512], f32, tag="p1", name="wps")
            for _ in range(24):
                nc.tensor.matmul(wps[:, 0:512], ones[:], wt[:],
                                 start=True, stop=True,
                                 skip_group_check=True)
            # dummy exp: pull the exp_and_others ACT table load (~2.7us)
            # forward, out of the first attention subgroup
            wx = xw.tile([1, 8], f32, tag="warmx")
            nc.vector.memset(wx[:], 0.0)
            wy = xw.tile([1, 8], f32, tag="warmy")
            nc.scalar.activation(wy[:], wx[:], AF.Exp)

            nev = [0]
            nsg = [0]

            def qk_group(ot_, half):
                # one o-tile (q_j or k_j), one token half (2 chunks)
                w1o = w1q if ot_ < 6 else w1k
                c0 = 128 * (ot_ % 6)
                dstt = (qkA if half == 0 else qkB)[ot_]
                for cc in range(2):
                    ch = 2 * half + cc
                    ps = pp1.tile([128, 512], f32, tag="p1", name="p1")
                    for kt in range(6):
                        nc.tensor.matmul(
                            ps[:, 0:CH],
                            w1o[kt][:, c0:c0 + 128],
                            xtok(kt, CH * ch, CH * (ch + 1)),
                            start=(kt == 0), stop=(kt == 5),
                        )
                    dst = dstt[:, CH * cc:CH * (cc + 1)]
                    if nev[0] % 2 == 0:
                        nc.scalar.activation(dst, ps[:, 0:CH], AF.Identity,
                                             bias=qkvb[:, ot_:ot_ + 1])
                    else:
                        nc.vector.tensor_scalar_add(dst, ps[:, 0:CH],
                                                    qkvb[:, ot_:ot_ + 1])
                    nev[0] += 1

            def emit_v(b):
                t0 = NTOK * b
                for k in range(2):
                    m = KT0 if k == 0 else KT1
                    ts_ = t0 + 128 * k
                    for c2 in range(2):
                        ps = pp1.tile([128, 512], f32, tag="p1", name="p1")
                        for kt in range(6):
                            nc.tensor.matmul(
                                ps[0:m, 0:VCH],
                                xtok(kt, ts_, ts_ + m),
                                w1v[kt][:, VCH * c2:VCH * (c2 + 1)],
                                start=(kt == 0), stop=(kt == 5),
                            )
                        src_ = ps[0:m, 0:VCH].rearrange("p (a b) -> p a b",
                                                        a=6)
                        dst = vn[b][k][0:m, 6 * c2:6 * (c2 + 1), 0:64]
                        if nev[0] % 2 == 0:
                            nc.scalar.activation(dst, src_, AF.Copy)
                        else:
                            nc.vector.tensor_copy(dst, src_)
                        nev[0] += 1

            def attn(b, j):
                # one attention subgroup: head pair (2j, 2j+1) of batch b
                qk = qkA if b < 4 else qkB
                t0_ = NTOK * b - (0 if b < 4 else 2 * CH)
                tg_ = NTOK * b
                pair = (2 * j, 2 * j + 1)
                psS = pS.tile([128, 2, 512], f32, tag="psS", name="psS")
                for i, h in enumerate(pair):
                    r0 = 64 * i
                    q_ap = qk[j][r0:r0 + 64, t0_:t0_ + NTOK]
                    nc.tensor.matmul(
                        psS[:, i, 0:NTOK],
                        qk[6 + j][r0:r0 + 64, t0_:t0_ + KT0],
                        q_ap,
                        start=True, stop=False, skip_group_check=True,
                    )
                    nc.tensor.matmul(
                        psS[0:KT1, i, NTOK:2 * NTOK],
                        qk[6 + j][r0:r0 + 64, t0_ + KT0:t0_ + NTOK],
                        q_ap,
                        start=False, stop=True, skip_group_check=True,
                    )
                u2e = upool.tile([128, 2, 2 * NTOK], bf16, tag="u2e",
                                 name="u2e")
                nc.scalar.activation(u2e[:], psS[:, :, 0:2 * NTOK], AF.Exp)
                u2 = upool.tile([128, 2, 2 * NTOK], bf16, tag="u2", name="u2")
                nc.vector.tensor_mul(u2[:], u2e[:], bT[:, 2 * j:2 * j + 2, :])
                psOD = pOD.tile([128, 512], f32, tag="psOD", name="psOD")
                # one bank: P@V head i -> partitions 64i:64i+64 free 0:197,
                # denominator (ones-lhsT, broadcast over partitions) at free
                # 256:453.  start=True marks the full bank row pending-zero
                # for the matmul's OWN partition range only, so the first
                # matmul of each 64-partition range carries start=True.
                for i, h in enumerate(pair):
                    nc.tensor.matmul(
                        psOD[64 * i:64 * i + 64, 0:NTOK],
                        vn[b][0][:, h, :],
                        u2[:, i, 0:NTOK],
                        start=True, stop=False, skip_group_check=True,
                    )
                for i in range(2):
                    nc.tensor.matmul(
                        psOD[64 * i:64 * i + 64, 256:256 + NTOK],
                        ones[:, 64 * i:64 * i + 64],
                        u2[:, i, 0:NTOK],
                        start=False, stop=False, skip_group_check=True,
                    )
                for i, h in enumerate(pair):
                    nc.tensor.matmul(
                        psOD[64 * i:64 * i + 64, 0:NTOK],
                        vn[b][1][0:KT1, h, :],
                        u2[0:KT1, i, NTOK:2 * NTOK],
                        start=False, stop=False, skip_group_check=True,
                    )
                for i in range(2):
                    nc.tensor.matmul(
                        psOD[64 * i:64 * i + 64, 256:256 + NTOK],
                        ones[0:KT1, 64 * i:64 * i + 64],
                        u2[0:KT1, i, NTOK:2 * NTOK],
                        start=False, stop=(i == 1), skip_group_check=True,
                    )
                rn = rnpool.tile([128, NTOK], f32, tag="rn", name="rn")
                nc.vector.reciprocal_approx_fast(
                    out=rn[:], in_=psOD[:, 256:256 + NTOK])
                nc.vector.tensor_mul(
                    OT[j][:, tg_:tg_ + NTOK], psOD[:, 0:NTOK], rn[:])
                nsg[0] += 1

            def proj_pair(pb):
                # one 394-wide column chunk (= batch pair (2pb, 2pb+1)) of
                # the projection
                c0 = 2 * NTOK * pb
                for co in range(6):
                    ps = pp1.tile([128, 512], f32, tag="p1", name="p1")
                    for ci in range(6):
                        nc.tensor.matmul(
                            ps[:, 0:2 * NTOK],
                            w2sb[ci][:, 128 * co:128 * co + 128],
                            OT[ci][:, c0:c0 + 2 * NTOK],
                            start=(ci == 0), stop=(ci == 5),
                        )
                    yst = ypool.tile([128, 2 * NTOK], f32, tag="yst",
                                     name="yst")
                    if (co + pb) % 2 == 0:
                        nc.scalar.activation(yst[:], ps[:, 0:2 * NTOK],
                                             AF.Copy)
                    else:
                        nc.vector.tensor_copy(yst[:], ps[:, 0:2 * NTOK])
                    nc.sync.dma_start(
                        out=yT_d[128 * co:128 * (co + 1), c0:c0 + 2 * NTOK],
                        in_=yst[:],
                    )

            # ---------------- interleaved schedule ----------------
            partA = [
                [(0, 0), (1, 0)],
                [(2, 0), (3, 0), (0, 1), (1, 1)],
                [(2, 1), (3, 1), (0, 2), (1, 2)],
                [(2, 2), (3, 2), (0, 3), (1, 3)],
                [(2, 3), (3, 3), (0, 4), (1, 4)],
                [(2, 4), (3, 4), (0, 5), (1, 5)],
            ]
            for r in range(6):
                # vn[4..7] is first read in part B (~halfway through), so
                # those v batches move past round 2 -- round 2 then has
                # weight-only work while the x half-B DMA is still landing
                if r in (0, 1, 3, 4):
                    vb0 = 2 * r if r < 2 else 2 * (r - 1)
                    emit_v(vb0)
                    emit_v(vb0 + 1)
                qk_group(r, 0)
                qk_group(6 + r, 0)
                for b_, j_ in partA[r]:
                    attn(b_, j_)
            partB = [
                [(2, 5), (3, 5), (4, 0), (5, 0)],
                [(6, 0), (7, 0), (4, 1), (5, 1)],
                [(6, 1), (7, 1), (4, 2), (5, 2)],
                [(6, 2), (7, 2), (4, 3), (5, 3)],
                [(6, 3), (7, 3), (4, 4), (5, 4)],
                [(6, 4), (7, 4), (4, 5), (5, 5)],
            ]
            for r in range(6):
                qk_group(r, 1)
                qk_group(6 + r, 1)
                for b_, j_ in partB[r]:
                    attn(b_, j_)
                if r == 0:
                    proj_pair(0)
                elif r == 1:
                    proj_pair(1)
            proj_pair(2)
            attn(6, 5)
            attn(7, 5)
            proj_pair(3)
    return nc


def build_nc():
    if "nc" not in _cache:
        from concourse import bacc
        nc = bacc.Bacc(None, target_bir_lowering=False, debug=False)
        _emit(nc)
        nc.compile()
        _cache["nc"] = nc
    return _cache["nc"]


def host_prep(x, qkv_w, q_bias, v_bias, rel_table, proj_w, proj_b, rel_index):
    """Shard + lay out inputs for the 8 cores. Returns list of in_maps."""
    x = np.asarray(x, np.float32)
    qkv_w = np.asarray(qkv_w, np.float32)
    q_bias = np.asarray(q_bias, np.float32)
    rel_table = np.asarray(rel_table, np.float32)
    rel_index = np.asarray(rel_index)

    sv = np.ones((3 * DIM, 1), np.float32)
    sv[:DIM] = SCALE
    w1 = np.ascontiguousarray((qkv_w * sv).T).astype(BF16)        # (768, 2304)
    # per-partition bias for the q,k o-tiles (k bias is zero by construction;
    # v_bias is added host-side: softmax rows sum to 1)
    qb = np.concatenate([q_bias * SCALE, np.zeros(DIM, np.float32)])
    qkvb = np.ascontiguousarray(qb.reshape(12, 128).T).astype(np.float32)

    bias = rel_table[rel_index]                # (197, 197, H), [q, k, h]
    BT = np.exp(bias.transpose(2, 1, 0))       # exp(bias): (H, k, q)
    bTdev = np.ones((128, H, 2 * NTOK), np.float32)
    bTdev[:, :, 0:NTOK] = BT.transpose(1, 0, 2)[0:128]
    bTdev[0:KT1, :, NTOK:2 * NTOK] = BT.transpose(1, 0, 2)[128:NTOK]
    bTdev = bTdev.astype(BF16)

    w2 = np.ascontiguousarray(proj_w.T).astype(BF16)              # (768, 768)

    in_maps = []
    for c in range(NCORES):
        xl = x[BL * c:BL * (c + 1)].reshape(T, DIM)
        xTc = np.ascontiguousarray(xl.T).astype(BF16)
        in_maps.append({
            "xT": xTc, "w1": w1, "qkvb": qkvb, "bT": bTdev, "w2": w2,
        })
    return in_maps


def run_device(in_maps, trace=False, tmpdir=None):
    from concourse.bass_utils import run_bass_kernel_spmd
    nc = build_nc()
    res = run_bass_kernel_spmd(
        nc, in_maps, core_ids=list(range(NCORES)), trace=trace, tmpdir=tmpdir
    )
    return res


def kernel(x, qkv_w, q_bias, v_bias, rel_table, proj_w, proj_b, rel_index):
    in_maps = host_prep(x, qkv_w, q_bias, v_bias, rel_table, proj_w, proj_b,
                        rel_index)
    res = run_device(in_maps)
    y = np.empty((B, NTOK, DIM), np.float32)
    for c in range(NCORES):
        yTc = res.results[c]["yT"]
        y[BL * c:BL * (c + 1)] = yTc.T.reshape(BL, NTOK, DIM)
    # exact host-side constant terms: attn rows sum to 1, so v_bias maps to
    # a constant (v_bias @ proj_w.T); proj_b is a plain add.
    v_bias = np.asarray(v_bias, np.float32)
    proj_b = np.asarray(proj_b, np.float32)
    const = proj_b.copy()
    if np.any(v_bias):
        const = const + v_bias @ np.asarray(proj_w, np.float32).T
    if np.any(const):
        y += const
    return y


# revision 32
# speedup vs baseline: 1.2058x; 1.0115x over previous
# BEiT-style windowed attention (B=64, N=197, C=768, H=12) on 8 Trainium2
# NeuronCores, data-parallel over batch (8 batches per core).
#
# Single interleaved pipeline per core (no phase barrier): qkv projection
# matmul groups, attention subgroups, and output-projection chunks are
# emitted round-robin so the ACT/DVE softmax work overlaps the dense
# matmuls and the PE is the only near-saturated engine.
#
#   round r (part A): v batches 2r, 2r+1 (r<4); q_r/k_r chunk-half A
#     (token cols 0:788 = batches 0-3); attention subgroups (b<4, j<=r)
#     as they become ready.
#   part B rounds: q_r/k_r chunk-half B (batches 4-7); remaining
#     subgroups; proj chunks per batch-pair as pairs complete.
#
# Attention subgroup (b, head-pair j): S.T = k.T q (2 heads row-tiled
# concurrently in the PE array), exp on ACT, u2 = exp(S)*exp(bias) on
# DVE (bias table sent as exp(bias) from host), then P@V col-tiled
# (head i -> psum partitions 64i:64i+64, free 0:197) plus an
# all-ones-lhsT matmul that writes the softmax denominator broadcast
# across partitions (free 256:453) in the same bank; DVE
# reciprocal_approx_fast [128,197] + one tensor_mul -> OT.
#
# PSUM: qkv/proj groups share one 2-buf single-bank pool (2), psS 2x2
# banks (4), psOD 2x1 (2) = 8 banks.
#
# Host side shards/transposes inputs, gathers exp(rel_table[rel_index]),
# scales q by SCALE (folded into W1/q_bias), and unshards the output.
# v_bias and proj_b are exact host-side constant adds (softmax rows sum
# to 1).

import numpy as np
import ml_dtypes

BF16 = ml_dtypes.bfloat16
E3M4 = ml_dtypes.float8_e3m4
WS = 128.0

DIM = 768
H = 12
HD = 64
NTOK = 197
B = 64
NCORES = 8
BL = B // NCORES          # batches per core = 8
T = BL * NTOK             # 1576 tokens per core
SCALE = HD ** -0.5
CH = 394                  # free-dim chunk for the dense matmuls (4*394 = 1576)
KT0, KT1 = 128, NTOK - 128   # key-token tile sizes (128, 69)
VCH = 384                 # v output-channel chunk (2*384 = 768)

_cache = {}


def _emit(nc):
    import concourse.mybir as mybir
    import concourse.tile as tile

    f32 = mybir.dt.float32
    bf16 = mybir.dt.bfloat16
    fp8 = mybir.dt.float8e3
    AF = mybir.ActivationFunctionType

    xT_d = nc.declare_dram_parameter("xT", [DIM, T], bf16, isOutput=False)
    w1_d = nc.declare_dram_parameter("w1", [DIM, 3 * DIM], fp8, isOutput=False)
    qkvb_d = nc.declare_dram_parameter("qkvb", [128, 12], f32, isOutput=False)
    bT_d = nc.declare_dram_parameter("bT", [128, H, 2 * NTOK], bf16, isOutput=False)
    w2_d = nc.declare_dram_parameter("w2", [DIM, DIM], bf16, isOutput=False)
    yT_d = nc.declare_dram_parameter("yT", [DIM, T], f32, isOutput=True)

    with tile.TileContext(nc) as tc:
        with (
            tc.tile_pool(name="const", bufs=1) as cpool,
            tc.tile_pool(name="qk", bufs=1) as qkpool,
            tc.tile_pool(name="vn", bufs=1) as vpool,
            tc.tile_pool(name="ot", bufs=1) as otpool,
            tc.tile_pool(name="xw", bufs=1) as xw,
            tc.tile_pool(name="p1", bufs=2, space="PSUM") as pp1,
            tc.tile_pool(name="pS", bufs=2, space="PSUM") as pS,
            tc.tile_pool(name="pOD", bufs=2, space="PSUM") as pOD,
            tc.tile_pool(name="u2", bufs=4) as upool,
            tc.tile_pool(name="rn", bufs=4) as rnpool,
            tc.tile_pool(name="yst", bufs=4) as ypool,
        ):
            ones = cpool.tile([128, 128], bf16, tag="ones")
            nc.gpsimd.memset(ones[:], 1.0)
            qkvb = cpool.tile([128, 12], f32, tag="qkvb")
            bT = cpool.tile([128, H, 2 * NTOK], bf16, tag="bT")
            w2sb = [cpool.tile([128, DIM], bf16, name=f"w2_{i}",
                               tag=f"w2_{i}") for i in range(6)]

            # q,k channels-on-partition, split by token half so early
            # batches' attention doesn't wait on late chunks
            qkA = [qkpool.tile([128, 2 * CH], bf16, name=f"qka{i}",
                               tag=f"qka{i}") for i in range(12)]
            qkB = [qkpool.tile([128, 2 * CH], bf16, name=f"qkb{i}",
                               tag=f"qkb{i}") for i in range(12)]
            # v in natural layout per (batch, key-tile): [kt, head, 64]
            vn = [[vpool.tile([128, H, HD], bf16, name=f"vn{b}_{k}",
                             tag=f"vn{b}_{k}")
                   for k in range(2)] for b in range(BL)]
            OT = [otpool.tile([128, T], bf16, name=f"ot{i}", tag=f"ot{i}")
                  for i in range(6)]

            xA = [xw.tile([128, 2 * CH], bf16, name=f"xa{i}",
                          tag=f"xa{i}") for i in range(6)]
            xB = [xw.tile([128, 2 * CH], bf16, name=f"xb{i}",
                          tag=f"xb{i}") for i in range(6)]
            w1q = [xw.tile([128, DIM], fp8, name=f"w1q{i}",
                           tag=f"w1q{i}") for i in range(6)]
            w1k = [xw.tile([128, DIM], fp8, name=f"w1k{i}",
                           tag=f"w1k{i}") for i in range(6)]
            w1v = [xw.tile([128, DIM], fp8, name=f"w1v{i}",
                           tag=f"w1v{i}") for i in range(6)]

            def xtok(kt, lo, hi):
                # x slice for token range [lo, hi) (never straddles 788)
                if hi <= 2 * CH:
                    return xA[kt][:, lo:hi]
                return xB[kt][:, lo - 2 * CH:hi - 2 * CH]

            # DMAs in first-use order: v batches run first (x half A +
            # w1 v-cols), then q0/k0 (w1 q/k cols), exp-bias table, x
            # half B, proj weights.
            qs = [nc.sync, nc.scalar, nc.gpsimd]
            nc.sync.dma_start(out=qkvb[:], in_=qkvb_d[:])
            for i in range(6):
                qs[i % 3].dma_start(out=xA[i][:],
                                    in_=xT_d[128 * i:128 * (i + 1), 0:2 * CH])
            for i in range(6):
                qs[i % 3].dma_start(out=w1v[i][:],
                                    in_=w1_d[128 * i:128 * (i + 1),
                                             2 * DIM:3 * DIM])
            for i in range(6):
                qs[i % 3].dma_start(out=w1q[i][:],
                                    in_=w1_d[128 * i:128 * (i + 1), 0:DIM])
            for i in range(6):
                qs[i % 3].dma_start(out=w1k[i][:],
                                    in_=w1_d[128 * i:128 * (i + 1),
                                             DIM:2 * DIM])
            nc.scalar.dma_start(out=bT[:], in_=bT_d[:])
            for i in range(6):
                qs[i % 3].dma_start(out=xB[i][:],
                                    in_=xT_d[128 * i:128 * (i + 1), 2 * CH:T])
            for i in range(6):
                qs[i % 3].dma_start(
                    out=w2sb[i][:], in_=w2_d[128 * i:128 * (i + 1), :])

            # PE warm-up: matmuls on a zeroed tile keep the HAM activity
            # window busy until real data lands (first v matmul ~8us), so
            # the pipeline starts at 2.4GHz instead of 1.2GHz.
            wt = xw.tile([128, 512], bf16, tag="warm")
            nc.vector.memset(wt[:], 0.0)
            wps = pp1.tile([128, 512], f32, tag="p1", name="wps")
            for _ in range(28):
                nc.tensor.matmul(wps[:, 0:512], ones[:], wt[:],
                                 start=True, stop=True,
                                 skip_group_check=True)
            # dummy exp: pull the exp_and_others ACT table load (~2.7us)
            # forward, out of the first attention subgroup
            wx = xw.tile([1, 8], f32, tag="warmx")
            nc.vector.memset(wx[:], 0.0)
            wy = xw.tile([1, 8], f32, tag="warmy")
            nc.scalar.activation(wy[:], wx[:], AF.Exp)

            nev = [0]
            nsg = [0]

            def qk_group(ot_, half):
                # one o-tile (q_j or k_j), one token half (2 chunks)
                w1o = w1q if ot_ < 6 else w1k
                c0 = 128 * (ot_ % 6)
                dstt = (qkA if half == 0 else qkB)[ot_]
                for cc in range(2):
                    ch = 2 * half + cc
                    ps = pp1.tile([128, 512], f32, tag="p1", name="p1")
                    for kt in range(6):
                        nc.tensor.matmul(
                            ps[:, 0:CH],
                            w1o[kt][:, c0:c0 + 128],
                            xtok(kt, CH * ch, CH * (ch + 1)),
                            start=(kt == 0), stop=(kt == 5),
                        )
                    dst = dstt[:, CH * cc:CH * (cc + 1)]
                    # w1 is stored as 128*w (fp8): psum holds 128*q_raw /
                    # 128*k_raw.  q gets SCALE/128^2 here (both 1/128
                    # factors of the S product folded into q); k is stored
                    # as 128*k_raw unscaled.
                    if ot_ < 6:
                        nc.scalar.activation(dst, ps[:, 0:CH], AF.Identity,
                                             bias=qkvb[:, ot_:ot_ + 1],
                                             scale=float(SCALE / (WS * WS)))
                    elif nev[0] % 2 == 0:
                        nc.scalar.activation(dst, ps[:, 0:CH], AF.Identity,
                                             bias=qkvb[:, ot_:ot_ + 1])
                    else:
                        nc.vector.tensor_scalar_add(dst, ps[:, 0:CH],
                                                    qkvb[:, ot_:ot_ + 1])
                    nev[0] += 1

            def emit_v(b):
                t0 = NTOK * b
                for k in range(2):
                    m = KT0 if k == 0 else KT1
                    ts_ = t0 + 128 * k
                    for c2 in range(2):
                        ps = pp1.tile([128, 512], f32, tag="p1", name="p1")
                        for kt in range(6):
                            nc.tensor.matmul(
                                ps[0:m, 0:VCH],
                                xtok(kt, ts_, ts_ + m),
                                w1v[kt][:, VCH * c2:VCH * (c2 + 1)],
                                start=(kt == 0), stop=(kt == 5),
                            )
                        src_ = ps[0:m, 0:VCH].rearrange("p (a b) -> p a b",
                                                        a=6)
                        dst = vn[b][k][0:m, 6 * c2:6 * (c2 + 1), 0:64]
                        if nev[0] % 2 == 0:
                            nc.scalar.activation(dst, src_, AF.Copy)
                        else:
                            nc.vector.tensor_copy(dst, src_)
                        nev[0] += 1

            def attn(b, j):
                # one attention subgroup: head pair (2j, 2j+1) of batch b
                qk = qkA if b < 4 else qkB
                t0_ = NTOK * b - (0 if b < 4 else 2 * CH)
                tg_ = NTOK * b
                pair = (2 * j, 2 * j + 1)
                psS = pS.tile([128, 2, 512], f32, tag="psS", name="psS")
                for i, h in enumerate(pair):
                    r0 = 64 * i
                    q_ap = qk[j][r0:r0 + 64, t0_:t0_ + NTOK]
                    nc.tensor.matmul(
                        psS[:, i, 0:NTOK],
                        qk[6 + j][r0:r0 + 64, t0_:t0_ + KT0],
                        q_ap,
                        start=True, stop=False, skip_group_check=True,
                    )
                    nc.tensor.matmul(
                        psS[0:KT1, i, NTOK:2 * NTOK],
                        qk[6 + j][r0:r0 + 64, t0_ + KT0:t0_ + NTOK],
                        q_ap,
                        start=False, stop=True, skip_group_check=True,
                    )
                u2e = upool.tile([128, 2, 2 * NTOK], bf16, tag="u2e",
                                 name="u2e")
                nc.scalar.activation(u2e[:], psS[:, :, 0:2 * NTOK], AF.Exp)
                u2 = upool.tile([128, 2, 2 * NTOK], bf16, tag="u2", name="u2")
                nc.vector.tensor_mul(u2[:], u2e[:], bT[:, 2 * j:2 * j + 2, :])
                psOD = pOD.tile([128, 512], f32, tag="psOD", name="psOD")
                # one bank: P@V head i -> partitions 64i:64i+64 free 0:197,
                # denominator (ones-lhsT, broadcast over partitions) at free
                # 256:453.  start=True marks the full bank row pending-zero
                # for the matmul's OWN partition range only, so the first
                # matmul of each 64-partition range carries start=True.
                for i, h in enumerate(pair):
                    nc.tensor.matmul(
                        psOD[64 * i:64 * i + 64, 0:NTOK],
                        vn[b][0][:, h, :],
                        u2[:, i, 0:NTOK],
                        start=True, stop=False, skip_group_check=True,
                    )
                for i in range(2):
                    nc.tensor.matmul(
                        psOD[64 * i:64 * i + 64, 256:256 + NTOK],
                        ones[:, 64 * i:64 * i + 64],
                        u2[:, i, 0:NTOK],
                        start=False, stop=False, skip_group_check=True,
                    )
                for i, h in enumerate(pair):
                    nc.tensor.matmul(
                        psOD[64 * i:64 * i + 64, 0:NTOK],
                        vn[b][1][0:KT1, h, :],
                        u2[0:KT1, i, NTOK:2 * NTOK],
                        start=False, stop=False, skip_group_check=True,
                    )
                for i in range(2):
                    nc.tensor.matmul(
                        psOD[64 * i:64 * i + 64, 256:256 + NTOK],
                        ones[0:KT1, 64 * i:64 * i + 64],
                        u2[0:KT1, i, NTOK:2 * NTOK],
                        start=False, stop=(i == 1), skip_group_check=True,
                    )
                rn = rnpool.tile([128, NTOK], f32, tag="rn", name="rn")
                nc.vector.reciprocal_approx_fast(
                    out=rn[:], in_=psOD[:, 256:256 + NTOK])
                nc.vector.tensor_mul(
                    OT[j][:, tg_:tg_ + NTOK], psOD[:, 0:NTOK], rn[:])
                nsg[0] += 1

            def proj_pair(pb):
                # one 394-wide column chunk (= batch pair (2pb, 2pb+1)) of
                # the projection
                c0 = 2 * NTOK * pb
                for co in range(6):
                    ps = pp1.tile([128, 512], f32, tag="p1", name="p1")
                    for ci in range(6):
                        nc.tensor.matmul(
                            ps[:, 0:2 * NTOK],
                            w2sb[ci][:, 128 * co:128 * co + 128],
                            OT[ci][:, c0:c0 + 2 * NTOK],
                            start=(ci == 0), stop=(ci == 5),
                        )
                    yst = ypool.tile([128, 2 * NTOK], f32, tag="yst",
                                     name="yst")
                    if (co + pb) % 2 == 0:
                        nc.scalar.activation(yst[:], ps[:, 0:2 * NTOK],
                                             AF.Copy)
                    else:
                        nc.vector.tensor_copy(yst[:], ps[:, 0:2 * NTOK])
                    nc.sync.dma_start(
                        out=yT_d[128 * co:128 * (co + 1), c0:c0 + 2 * NTOK],
                        in_=yst[:],
                    )

            # ---------------- interleaved schedule ----------------
            partA = [
                [(0, 0), (1, 0)],
                [(2, 0), (3, 0), (0, 1), (1, 1)],
                [(2, 1), (3, 1), (0, 2), (1, 2)],
                [(2, 2), (3, 2), (0, 3), (1, 3)],
                [(2, 3), (3, 3), (0, 4), (1, 4)],
                [(2, 4), (3, 4), (0, 5), (1, 5)],
            ]
            for r in range(6):
                # vn[4..7] is first read in part B (~halfway through), so
                # those v batches move past round 2 -- round 2 then has
                # weight-only work while the x half-B DMA is still landing
                if r in (0, 1, 3, 4):
                    vb0 = 2 * r if r < 2 else 2 * (r - 1)
                    emit_v(vb0)
                    emit_v(vb0 + 1)
                qk_group(r, 0)
                qk_group(6 + r, 0)
                for b_, j_ in partA[r]:
                    attn(b_, j_)
            partB = [
                [(2, 5), (3, 5), (4, 0), (5, 0)],
                [(6, 0), (7, 0), (4, 1), (5, 1)],
                [(6, 1), (7, 1), (4, 2), (5, 2)],
                [(6, 2), (7, 2), (4, 3), (5, 3)],
                [(6, 3), (7, 3), (4, 4), (5, 4)],
                [(6, 4), (7, 4), (4, 5), (5, 5)],
            ]
            for r in range(6):
                qk_group(r, 1)
                qk_group(6 + r, 1)
                for b_, j_ in partB[r]:
                    attn(b_, j_)
                if r == 0:
                    proj_pair(0)
                elif r == 1:
                    proj_pair(1)
            proj_pair(2)
            attn(6, 5)
            attn(7, 5)
            proj_pair(3)
    return nc


def build_nc():
    if "nc" not in _cache:
        from concourse import bacc
        nc = bacc.Bacc(None, target_bir_lowering=False, debug=False)
        _emit(nc)
        nc.compile()
        _cache["nc"] = nc
    return _cache["nc"]


def host_prep(x, qkv_w, q_bias, v_bias, rel_table, proj_w, proj_b, rel_index):
    """Shard + lay out inputs for the 8 cores. Returns list of in_maps."""
    x = np.asarray(x, np.float32)
    qkv_w = np.asarray(qkv_w, np.float32)
    q_bias = np.asarray(q_bias, np.float32)
    rel_table = np.asarray(rel_table, np.float32)
    rel_index = np.asarray(rel_index)

    w1 = np.ascontiguousarray((qkv_w * WS).T).astype(E3M4)       # (768, 2304)
    # per-partition bias for the q,k o-tiles (k bias is zero by construction;
    # v_bias is added host-side: softmax rows sum to 1).  q carries
    # SCALE/128 net scale on device, so its bias does too.
    qb = np.concatenate([q_bias * SCALE / WS, np.zeros(DIM, np.float32)])
    qkvb = np.ascontiguousarray(qb.reshape(12, 128).T).astype(np.float32)

    bias = rel_table[rel_index]                # (197, 197, H), [q, k, h]
    BT = np.exp(bias.transpose(2, 1, 0))       # exp(bias): (H, k, q)
    bTdev = np.ones((128, H, 2 * NTOK), np.float32)
    bTdev[:, :, 0:NTOK] = BT.transpose(1, 0, 2)[0:128]
    bTdev[0:KT1, :, NTOK:2 * NTOK] = BT.transpose(1, 0, 2)[128:NTOK]
    bTdev = bTdev.astype(BF16)

    # v is stored as 128*v (fp8 w1v); fold the 1/128 into w2
    w2 = np.ascontiguousarray((proj_w / WS).T).astype(BF16)       # (768, 768)

    in_maps = []
    for c in range(NCORES):
        xl = x[BL * c:BL * (c + 1)].reshape(T, DIM)
        xTc = np.ascontiguousarray(xl.T).astype(BF16)
        in_maps.append({
            "xT": xTc, "w1": w1, "qkvb": qkvb, "bT": bTdev, "w2": w2,
        })
    return in_maps


def run_device(in_maps, trace=False, tmpdir=None):
    from concourse.bass_utils import run_bass_kernel_spmd
    nc = build_nc()
    res = run_bass_kernel_spmd(
        nc, in_maps, core_ids=list(range(NCORES)), trace=trace, tmpdir=tmpdir
    )
    return res


def kernel(x, qkv_w, q_bias, v_bias, rel_table, proj_w, proj_b, rel_index):
    in_maps = host_prep(x, qkv_w, q_bias, v_bias, rel_table, proj_w, proj_b,
                        rel_index)
    res = run_device(in_maps)
    y = np.empty((B, NTOK, DIM), np.float32)
    for c in range(NCORES):
        yTc = res.results[c]["yT"]
        y[BL * c:BL * (c + 1)] = yTc.T.reshape(BL, NTOK, DIM)
    # exact host-side constant terms: attn rows sum to 1, so v_bias maps to
    # a constant (v_bias @ proj_w.T); proj_b is a plain add.
    v_bias = np.asarray(v_bias, np.float32)
    proj_b = np.asarray(proj_b, np.float32)
    const = proj_b.copy()
    if np.any(v_bias):
        const = const + v_bias @ np.asarray(proj_w, np.float32).T
    if np.any(const):
        y += const
    return y
